# revision 1
# baseline (speedup 1.0000x reference)
"""Trainium2 Bass kernel for a dense transformer block, sharded over 8 NeuronCores.

Sharding: core c handles batch b=c//2 and half hf=c%2 of that batch's 2048
tokens ("own" tokens). K/V are computed for the full 2048-token batch on both
cores of a pair, so no collectives are needed. Inputs are prepared host-side
(transposed weights, rolled token axis), outputs gathered host-side.
"""

import numpy as np

from contextlib import ExitStack

import concourse.bass as bass
import concourse.bacc as bacc
import concourse.tile as tile
import concourse.mybir as mybir

F32 = mybir.dt.float32
F32R = mybir.dt.float32r
AF = mybir.ActivationFunctionType
OP = mybir.AluOpType

EPS = 1e-5


class Cfg:
    def __init__(self, E=1024, H=16, MLP=4096, T_OWN=1024, T_FULL=2048, repeat=1, act="gelu"):
        self.E, self.H, self.MLP = E, H, MLP
        self.T_OWN, self.T_FULL = T_OWN, T_FULL
        self.D = 64
        assert E == self.D * H
        self.NE = E // 128
        self.NM = MLP // 128
        self.NQB = T_OWN // 512
        self.NFB = T_FULL // 512
        self.NTK = T_FULL // 128
        self.G = 2
        self.HPG = H // self.G
        self.NKT_G = self.HPG * self.D // 128  # k tiles (head pairs) per group
        self.repeat = repeat
        self.act = act
        assert self.HPG * self.D % 128 == 0


def build(cfg: Cfg):
    E, MLP, T_OWN, T_FULL = cfg.E, cfg.MLP, cfg.T_OWN, cfg.T_FULL

    nc = bacc.Bacc("TRN2", target_bir_lowering=False, debug=False)

    d = {}
    d["xT"] = nc.dram_tensor("xT", [E, T_FULL], F32, kind="ExternalInput")
    d["qkvT"] = nc.dram_tensor("qkvT", [E, 3 * E], F32, kind="ExternalInput")
    d["fcT"] = nc.dram_tensor("fcT", [E, E], F32, kind="ExternalInput")
    d["w1T"] = nc.dram_tensor("w1T", [E, MLP], F32, kind="ExternalInput")
    d["w2T"] = nc.dram_tensor("w2T", [MLP, E], F32, kind="ExternalInput")
    d["ln1"] = nc.dram_tensor("ln1", [2, E], F32, kind="ExternalInput")
    d["ln2"] = nc.dram_tensor("ln2", [2, E], F32, kind="ExternalInput")
    d["fcb"] = nc.dram_tensor("fcb", [E], F32, kind="ExternalInput")
    d["b1"] = nc.dram_tensor("b1", [MLP], F32, kind="ExternalInput")
    d["b2"] = nc.dram_tensor("b2", [E], F32, kind="ExternalInput")
    d["ones"] = nc.dram_tensor("ones", [max(512, T_FULL)], F32, kind="ExternalInput")
    d["out"] = nc.dram_tensor("out", [E, T_OWN], F32, kind="ExternalOutput")
    d["h_dram"] = nc.dram_tensor("h_scratch", [E, T_FULL], F32)

    with tile.TileContext(nc) as tc, nc.allow_low_precision(
        reason="fp32r matmul inputs by design"
    ):
        if cfg.repeat == 1:
            _body(nc, tc, cfg, d)
        else:
            with tc.For_i(0, cfg.repeat, 1):
                _body(nc, tc, cfg, d)
    nc.compile()
    return nc


def _ln_stats(nc, tc, cfg, pools, src_fn, nblk, ones_col, eps_t, srst, snb):
    """Column stats over feature dim via ones-matmuls.

    src_fn(e, tb) -> [128,512] f32r AP of the input; writes rstd row into
    srst[0:1] and -mu*rstd into snb[0:1] (both f32r rows)."""
    E, NE = cfg.E, cfg.NE
    sq_pool, st_ps, row_pool = pools
    for tb in range(nblk):
        sl = slice(tb * 512, (tb + 1) * 512)
        s1 = st_ps.tile([1, 512], F32, tag="s1")
        s2 = st_ps.tile([1, 512], F32, tag="s2")
        for e in range(NE):
            src = src_fn(e, tb)
            sq = sq_pool.tile([128, 512], F32R, tag="sq")
            nc.vector.tensor_tensor(sq[:], src.bitcast(F32), src.bitcast(F32), OP.mult)
            nc.tensor.matmul(s1[:], ones_col[:], src, start=(e == 0), stop=(e == NE - 1))
            nc.tensor.matmul(s2[:], ones_col[:], sq[:], start=(e == 0), stop=(e == NE - 1))
        m_row = row_pool.tile([1, 512], F32, tag="mrow")
        nc.vector.tensor_scalar_mul(m_row[:], s1[:], 1.0 / E)
        v_row = row_pool.tile([1, 512], F32, tag="vrow")
        nc.vector.tensor_scalar_mul(v_row[:], s2[:], 1.0 / E)
        msq = row_pool.tile([1, 512], F32, tag="msq")
        nc.vector.tensor_tensor(msq[:], m_row[:], m_row[:], OP.mult)
        nc.vector.tensor_tensor(v_row[:], v_row[:], msq[:], OP.subtract)
        sd = row_pool.tile([1, 512], F32, tag="sd")
        nc.scalar.activation(sd[:], v_row[:], AF.Sqrt, bias=eps_t[:], scale=1.0)
        nc.vector.reciprocal(srst[0:1, sl], sd[:])
        nc.vector.scalar_tensor_tensor(
            snb[0:1, sl], m_row[:], -1.0, srst[0:1, sl].bitcast(F32),
            op0=OP.mult, op1=OP.mult)


def _ln_apply(nc, map_ps, gb_e, srst, snb, src_ap_f32, dst_ap, sl):
    """dst = src * (g x rstd) + (g x (-mu*rstd) + b x 1), all [128, 512]."""
    a_ps = map_ps.tile([128, 512], F32, tag="amap")
    nc.tensor.matmul(a_ps[:], gb_e[0:1, :], srst[0:1, sl], start=True, stop=True)
    b_ps = map_ps.tile([128, 512], F32, tag="bmap")
    nc.tensor.matmul(b_ps[:], gb_e[0:2, :], snb[0:2, sl], start=True, stop=True)
    nc.vector.tensor_tensor(dst_ap, src_ap_f32, a_ps[:], OP.mult)
    nc.vector.tensor_tensor(dst_ap, dst_ap.bitcast(F32), b_ps[:], OP.add)


def _body(nc, tc, cfg, d):
    E, H, MLP, D = cfg.E, cfg.H, cfg.MLP, cfg.D
    NE, NM, NQB, NFB, NTK = cfg.NE, cfg.NM, cfg.NQB, cfg.NFB, cfg.NTK
    T_OWN, T_FULL, G, HPG, NKT_G = cfg.T_OWN, cfg.T_FULL, cfg.G, cfg.HPG, cfg.NKT_G
    xT, qkvT, fcT, w1T, w2T = d["xT"], d["qkvT"], d["fcT"], d["w1T"], d["w2T"]
    ln1, ln2, fcb, b1, b2 = d["ln1"], d["ln2"], d["fcb"], d["b1"], d["b2"]
    ones, out, h_dram = d["ones"], d["out"], d["h_dram"]

    with ExitStack() as ctx:
        consts = ctx.enter_context(tc.tile_pool(name="consts", bufs=1))

        ones_col = consts.tile([128, 1], F32R)
        nc.sync.dma_start(ones_col[:], ones.ap()[0:128].rearrange("(p c) -> p c", c=1).bitcast(F32R))
        ones64_row = consts.tile([1, 64], F32R)
        nc.sync.dma_start(ones64_row[:], ones.ap()[0:64].unsqueeze(0).bitcast(F32R))

        gb1, gb2, fcb_c, b2_c, b1_c = [], [], [], [], []
        for e in range(NE):
            t1 = consts.tile([2, 128], F32R, tag=f"gb1_{e}")
            nc.sync.dma_start(t1[:], ln1.ap()[:, e * 128:(e + 1) * 128].bitcast(F32R))
            gb1.append(t1)
            t2 = consts.tile([2, 128], F32R, tag=f"gb2_{e}")
            nc.sync.dma_start(t2[:], ln2.ap()[:, e * 128:(e + 1) * 128].bitcast(F32R))
            gb2.append(t2)
            t = consts.tile([128, 1], F32, tag=f"fcb_{e}")
            nc.sync.dma_start(t[:], fcb.ap()[e * 128:(e + 1) * 128].rearrange("(p c) -> p c", c=1))
            fcb_c.append(t)
            t = consts.tile([128, 1], F32, tag=f"b2c_{e}")
            nc.sync.dma_start(t[:], b2.ap()[e * 128:(e + 1) * 128].rearrange("(p c) -> p c", c=1))
            b2_c.append(t)
        for m in range(NM):
            t = consts.tile([128, 1], F32, tag=f"b1c_{m}")
            nc.sync.dma_start(t[:], b1.ap()[m * 128:(m + 1) * 128].rearrange("(p c) -> p c", c=1))
            b1_c.append(t)
        eps_t = consts.tile([1, 1], F32)
        nc.vector.memset(eps_t[:], EPS)

        # long-lived attention output buffer (must sit below p1 pools in the stack)
        av_pool = ctx.enter_context(tc.tile_pool(name="avp", bufs=NE))
        av_tiles = [av_pool.tile([128, T_OWN], F32R, tag="avt", name="avt")
                    for _ in range(NE)]

        k_tiles, v_tiles = {}, {}
        q_tiles = []

        def kv_group(g, acc_ps, hs_pool, wk_pool, wv_pool):
            def h_mov(e, tb, tag):
                nb = NE + 2 if tag == "hsv" else 4
                t = hs_pool.tile([128, 512], F32R, tag=tag, name="hm", bufs=nb)
                nc.sync.dma_start(
                    t[:], h_dram.ap()[e * 128:(e + 1) * 128,
                                      tb * 512:(tb + 1) * 512].bitcast(F32R))
                return t[:]

            # K: stationary = resident weight tiles; h streamed once per (e, tb)
            wk = {}
            for dkt in range(NKT_G):
                for e in range(NE):
                    wt = wk_pool.tile([128, 128], F32R, tag=f"wk{dkt}_{e}", name="wk")
                    col0 = E + g * HPG * D + dkt * 128
                    nc.scalar.dma_start(
                        wt[:], qkvT.ap()[e * 128:(e + 1) * 128, col0:col0 + 128].bitcast(F32R))
                    wk[(dkt, e)] = wt
                k_tiles[(g, dkt)] = k_pool.tile([128, T_FULL], F32R, tag="kt", name="kt")
            wv = []
            for e in range(NE):
                wt = wv_pool.tile([128, HPG * D], F32R, tag="wv", name="wv")
                col0 = 2 * E + g * HPG * D
                nc.scalar.dma_start(
                    wt[:], qkvT.ap()[e * 128:(e + 1) * 128, col0:col0 + HPG * D].bitcast(F32R))
                wv.append(wt)
            for tb in range(NFB):
                hv = [h_mov(e, tb, "hsv") for e in range(NE)]
                ps = {dkt: acc_ps.tile([128, 512], F32, tag="acc", name="acc")
                      for dkt in range(NKT_G)}
                for e in range(NE):
                    for dkt in range(NKT_G):
                        nc.tensor.matmul(ps[dkt][:], wk[(dkt, e)][:], hv[e][:],
                                         start=(e == 0), stop=(e == NE - 1))
                for dkt in range(NKT_G):
                    nc.vector.tensor_copy(
                        k_tiles[(g, dkt)][:, tb * 512:(tb + 1) * 512], ps[dkt][:])
                for tki in range(512 // 128):
                    tk = tb * 4 + tki
                    vt = v_pool.tile([128, HPG, 65], F32R, tag="vt", name="vt")
                    v_tiles[(g, tk)] = vt
                    one_src = bass.AP(
                        tensor=ones.ap().tensor, offset=0,
                        ap=[[0, 128], [1, HPG], [1, 1]])
                    nc.sync.dma_start(vt[:, :, 64:65], one_src.bitcast(F32R))
                    ps_v = acc_ps.tile([128, HPG * D], F32, tag="accv", name="accv", bufs=2)
                    off = tki * 128
                    for e in range(NE):
                        nc.tensor.matmul(ps_v[:], hv[e][:, off:off + 128], wv[e][:],
                                         start=(e == 0), stop=(e == NE - 1))
                    nc.vector.tensor_copy(
                        vt[:, :, 0:64], ps_v[:].rearrange("p (h d) -> p h d", d=64))

        def attention_group(g):
            with ExitStack() as pa:
                sc_ps = pa.enter_context(tc.tile_pool(name=f"scps{g}", bufs=3, space="PSUM"))
                avg_ps = pa.enter_context(tc.tile_pool(name=f"avps{g}", bufs=3, space="PSUM"))
                rm_ps = pa.enter_context(tc.tile_pool(name=f"rmps{g}", bufs=1, space="PSUM"))
                ex_pool = pa.enter_context(tc.tile_pool(name=f"exp{g}", bufs=4))
                rec_pool = pa.enter_context(tc.tile_pool(name=f"rec{g}", bufs=4))

                for hp in range(NKT_G):
                    hpg = g * NKT_G + hp
                    for tqb in range(NQB):
                        qsl = slice(tqb * 512, (tqb + 1) * 512)
                        av_a = avg_ps.tile([65, 512], F32, tag="av", name="av")
                        av_b = avg_ps.tile([65, 512], F32, tag="av", name="av")
                        for tk in range(NTK):
                            ksl = slice(tk * 128, (tk + 1) * 128)
                            sc_a = sc_ps.tile([128, 512], F32, tag="sc", name="sc")
                            sc_b = sc_ps.tile([128, 512], F32, tag="sc", name="sc")
                            nc.tensor.matmul(sc_a[:], k_tiles[(g, hp)][0:64, ksl],
                                             q_tiles[hpg][0:64, qsl], start=True, stop=True)
                            nc.tensor.matmul(sc_b[:], k_tiles[(g, hp)][64:128, ksl],
                                             q_tiles[hpg][64:128, qsl], start=True, stop=True)
                            ex_a = ex_pool.tile([128, 512], F32R, tag="ex", name="ex")
                            nc.scalar.activation(ex_a[:], sc_a[:], AF.Exp)
                            ex_b = ex_pool.tile([128, 512], F32R, tag="ex", name="ex")
                            nc.scalar.activation(ex_b[:], sc_b[:], AF.Exp)
                            nc.tensor.matmul(av_a[:], v_tiles[(g, tk)][:, 2 * hp, :],
                                             ex_a[:], start=(tk == 0), stop=(tk == NTK - 1))
                            nc.tensor.matmul(av_b[:], v_tiles[(g, tk)][:, 2 * hp + 1, :],
                                             ex_b[:], start=(tk == 0), stop=(tk == NTK - 1))
                        for head, av_ps_t in ((0, av_a), (1, av_b)):
                            rrow = rec_pool.tile([1, 512], F32R, tag="rr", name="rr")
                            nc.vector.reciprocal(rrow[:], av_ps_t[64:65, :])
                            rmp = rm_ps.tile([64, 512], F32, tag="rm", name="rm")
                            nc.tensor.matmul(rmp[:], ones64_row[:], rrow[:], start=True, stop=True)
                            rms = rec_pool.tile([64, 512], F32, tag="rms", name="rms")
                            nc.vector.tensor_copy(rms[:], rmp[:])
                            nc.vector.tensor_tensor(
                                av_tiles[hpg][head * 64:(head + 1) * 64, qsl],
                                av_ps_t[0:64, :], rms[:], OP.mult)

        # ================= Phases 1-3 =================
        with ExitStack() as p1:
            q_pool = p1.enter_context(tc.tile_pool(name="qp", bufs=NE))
            k_pool = p1.enter_context(tc.tile_pool(name="kp", bufs=NKT_G))
            v_pool = p1.enter_context(tc.tile_pool(name="vp", bufs=NTK))
            q_tiles.extend(q_pool.tile([128, T_OWN], F32R, tag="qt", name="qt")
                           for _ in range(NE))

            # --- LN1: stats + normalize, streaming x; h goes to DRAM only ---
            with ExitStack() as pA:
                x_pool = pA.enter_context(tc.tile_pool(name="xp", bufs=4))
                sq_pool = pA.enter_context(tc.tile_pool(name="sqp", bufs=2))
                st_ps = pA.enter_context(tc.tile_pool(name="stps", bufs=1, space="PSUM"))
                row_pool = pA.enter_context(tc.tile_pool(name="rows", bufs=2))
                map_ps = pA.enter_context(tc.tile_pool(name="mapps", bufs=2, space="PSUM"))
                stat_pool = pA.enter_context(tc.tile_pool(name="statp", bufs=1))
                h_tmp = pA.enter_context(tc.tile_pool(name="htmp", bufs=4))
                srst1 = stat_pool.tile([1, T_FULL], F32R, tag="srst1")
                snb1 = stat_pool.tile([2, T_FULL], F32R, tag="snb1")
                nc.sync.dma_start(
                    snb1[1:2, :], ones.ap()[0:T_FULL].unsqueeze(0).bitcast(F32R))

                for tb in range(NFB):
                    sl = slice(tb * 512, (tb + 1) * 512)
                    xts = []
                    for e in range(NE):
                        t = x_pool.tile([128, 512], F32R, tag="xt", name="xt",
                                        bufs=NE + 2)
                        nc.sync.dma_start(
                            t[:], xT.ap()[e * 128:(e + 1) * 128, sl].bitcast(F32R))
                        xts.append(t)
                    _ln_stats(nc, tc, cfg, (sq_pool, st_ps, row_pool),
                              lambda e, _tb: xts[e][:], 1, ones_col, eps_t,
                              srst1[0:1, sl], snb1[0:2, sl])
                    for e in range(NE):
                        ht = h_tmp.tile([128, 512], F32R, tag="ht", name="ht")
                        _ln_apply(nc, map_ps, gb1[e], srst1, snb1,
                                  xts[e][:].bitcast(F32), ht[:], sl)
                        nc.sync.dma_start(
                            h_dram.ap()[e * 128:(e + 1) * 128, sl], ht[:].bitcast(F32))

            # --- Q projection (h streamed back from DRAM) ---
            with ExitStack() as pq:
                wq_pool = pq.enter_context(tc.tile_pool(name="wqp", bufs=3))
                hq_pool = pq.enter_context(tc.tile_pool(name="hqp", bufs=6))
                acc_ps = pq.enter_context(tc.tile_pool(name="accps", bufs=8, space="PSUM"))
                NJ = min(4 if NQB * 4 <= 8 else 2, NE)
                for dq4 in range(NE // NJ):
                    q_ps = {(j, tqb): acc_ps.tile([128, 512], F32, tag="acc", name="acc")
                            for j in range(NJ) for tqb in range(NQB)}
                    for e in range(NE):
                        wt = wq_pool.tile([128, NJ * 128], F32R, tag="wq")
                        nc.scalar.dma_start(
                            wt[:], qkvT.ap()[e * 128:(e + 1) * 128,
                                             dq4 * NJ * 128:(dq4 + 1) * NJ * 128].bitcast(F32R))
                        for tqb in range(NQB):
                            hq = hq_pool.tile([128, 512], F32R, tag="hq", name="hq")
                            nc.sync.dma_start(
                                hq[:], h_dram.ap()[e * 128:(e + 1) * 128,
                                                   tqb * 512:(tqb + 1) * 512].bitcast(F32R))
                            for j in range(NJ):
                                nc.tensor.matmul(
                                    q_ps[(j, tqb)][:], wt[:, j * 128:(j + 1) * 128],
                                    hq[:], start=(e == 0), stop=(e == NE - 1))
                    for j in range(NJ):
                        for tqb in range(NQB):
                            nc.vector.tensor_copy(
                                q_tiles[dq4 * NJ + j][:, tqb * 512:(tqb + 1) * 512],
                                q_ps[(j, tqb)][:])

            # --- K/V + attention per group ---
            for g in range(G):
                with ExitStack() as pb:
                    hs_pool = pb.enter_context(tc.tile_pool(name=f"hsp{g}", bufs=4))
                    wk_pool = pb.enter_context(tc.tile_pool(name=f"wkp{g}", bufs=1))
                    wv_pool = pb.enter_context(tc.tile_pool(name=f"wvp{g}", bufs=NE))
                    acc_ps = pb.enter_context(
                        tc.tile_pool(name=f"kvps{g}", bufs=4, space="PSUM"))
                    kv_group(g, acc_ps, hs_pool, wk_pool, wv_pool)
                attention_group(g)
        # q/k/v freed here

        # ================= Phase 4: fc_out + residual =================
        x2_pool = ctx.enter_context(tc.tile_pool(name="x2p", bufs=NE))
        x2_tiles = [x2_pool.tile([128, T_OWN], F32R, tag="x2t", name="x2t")
                    for _ in range(NE)]
        with ExitStack() as p4:
            wf_pool = p4.enter_context(tc.tile_pool(name="wfp", bufs=3))
            xr_pool = p4.enter_context(tc.tile_pool(name="xrp", bufs=3))
            fc_ps = p4.enter_context(tc.tile_pool(name="fcps", bufs=5, space="PSUM"))
            for o2 in range(NE // 2):
                ps = {(j, tqb): fc_ps.tile([128, 512], F32, tag="fc", name="fc")
                      for j in range(2) for tqb in range(NQB)}
                for e in range(NE):
                    wt = wf_pool.tile([128, 256], F32R, tag="wf")
                    nc.scalar.dma_start(
                        wt[:], fcT.ap()[e * 128:(e + 1) * 128,
                                        o2 * 256:(o2 + 1) * 256].bitcast(F32R))
                    for j in range(2):
                        for tqb in range(NQB):
                            nc.tensor.matmul(
                                ps[(j, tqb)][:], wt[:, j * 128:(j + 1) * 128],
                                av_tiles[e][:, tqb * 512:(tqb + 1) * 512],
                                start=(e == 0), stop=(e == NE - 1))
                for j in range(2):
                    o = o2 * 2 + j
                    for tqb in range(NQB):
                        sl = slice(tqb * 512, (tqb + 1) * 512)
                        xr = xr_pool.tile([128, 512], F32, tag="xr")
                        nc.sync.dma_start(xr[:], xT.ap()[o * 128:(o + 1) * 128, sl])
                        nc.vector.scalar_tensor_tensor(
                            x2_tiles[o][:, sl], ps[(j, tqb)][:], fcb_c[o][:], xr[:],
                            op0=OP.add, op1=OP.add)

        # ================= Phase 5: LN2 =================
        h2_pool = ctx.enter_context(tc.tile_pool(name="h2p", bufs=NE))
        h2_tiles = [h2_pool.tile([128, T_OWN], F32R, tag="h2t", name="h2t")
                    for _ in range(NE)]
        with ExitStack() as p5:
            sq_pool = p5.enter_context(tc.tile_pool(name="sq2p", bufs=2))
            st_ps = p5.enter_context(tc.tile_pool(name="st2ps", bufs=1, space="PSUM"))
            row_pool = p5.enter_context(tc.tile_pool(name="rows2", bufs=2))
            map_ps = p5.enter_context(tc.tile_pool(name="map2ps", bufs=2, space="PSUM"))
            stat2_pool = p5.enter_context(tc.tile_pool(name="stat2p", bufs=1))
            srst2 = stat2_pool.tile([1, T_OWN], F32R, tag="srst2")
            snb2 = stat2_pool.tile([2, T_OWN], F32R, tag="snb2")
            nc.sync.dma_start(snb2[1:2, :], ones.ap()[0:T_OWN].unsqueeze(0).bitcast(F32R))
            _ln_stats(nc, tc, cfg, (sq_pool, st_ps, row_pool),
                      lambda e, tb: x2_tiles[e][:, tb * 512:(tb + 1) * 512],
                      NQB, ones_col, eps_t, srst2, snb2)
            for tb in range(NQB):
                sl = slice(tb * 512, (tb + 1) * 512)
                for e in range(NE):
                    _ln_apply(nc, map_ps, gb2[e], srst2, snb2,
                              x2_tiles[e][:, sl].bitcast(F32), h2_tiles[e][:, sl], sl)

        # ================= Phase 6: MLP =================
        for tqb in range(NQB):
            sl = slice(tqb * 512, (tqb + 1) * 512)
            with ExitStack() as p6:
                g_pool = p6.enter_context(tc.tile_pool(name=f"gp{tqb}", bufs=NM))
                g_tiles = []
                with ExitStack() as p6a:
                    w1_pool = p6a.enter_context(tc.tile_pool(name=f"w1p{tqb}", bufs=3))
                    m1_ps = p6a.enter_context(
                        tc.tile_pool(name=f"m1ps{tqb}", bufs=6, space="PSUM"))
                    for m4 in range(NM // 4):
                        ps = {j: m1_ps.tile([128, 512], F32, tag="m1", name="m1")
                              for j in range(4)}
                        for e in range(NE):
                            wt = w1_pool.tile([128, 512], F32R, tag="w1")
                            nc.scalar.dma_start(
                                wt[:], w1T.ap()[e * 128:(e + 1) * 128,
                                                m4 * 512:(m4 + 1) * 512].bitcast(F32R))
                            for j in range(4):
                                nc.tensor.matmul(ps[j][:], wt[:, j * 128:(j + 1) * 128],
                                                 h2_tiles[e][:, sl],
                                                 start=(e == 0), stop=(e == NE - 1))
                        for j in range(4):
                            gt = g_pool.tile([128, 512], F32R, tag="gt", name="gt")
                            nc.scalar.activation(gt[:], ps[j][:],
                                                 AF.Gelu if cfg.act == "gelu" else AF.Tanh,
                                                 bias=b1_c[m4 * 4 + j][:], scale=1.0)
                            g_tiles.append(gt)

                with ExitStack() as p6b:
                    w2_pool = p6b.enter_context(tc.tile_pool(name=f"w2p{tqb}", bufs=3))
                    out_pool = p6b.enter_context(tc.tile_pool(name=f"op{tqb}", bufs=3))
                    m2_ps = p6b.enter_context(
                        tc.tile_pool(name=f"m2ps{tqb}", bufs=NE, space="PSUM"))
                    ps = {o: m2_ps.tile([128, 512], F32, tag="m2", name="m2")
                          for o in range(NE)}
                    for m in range(NM):
                        wt = w2_pool.tile([128, E], F32R, tag="w2")
                        nc.scalar.dma_start(
                            wt[:], w2T.ap()[m * 128:(m + 1) * 128, :].bitcast(F32R))
                        for o in range(NE):
                            nc.tensor.matmul(ps[o][:], wt[:, o * 128:(o + 1) * 128],
                                             g_tiles[m][:],
                                             start=(m == 0), stop=(m == NM - 1))
                    for o in range(NE):
                        ot = out_pool.tile([128, 512], F32, tag="ot", name="ot")
                        nc.vector.scalar_tensor_tensor(
                            ot[:], ps[o][:], b2_c[o][:], x2_tiles[o][:, sl].bitcast(F32),
                            op0=OP.add, op1=OP.add)
                        nc.sync.dma_start(out.ap()[o * 128:(o + 1) * 128, sl], ot[:])



# ----------------------------------------------------------------------------
# host driver
# ----------------------------------------------------------------------------
B, S, E_FULL, H_FULL, MLP_FULL = 4, 2048, 1024, 16, 4096
_cache = {}


def _get_nc():
    if "nc" not in _cache:
        _cache["nc"] = build(Cfg())
    return _cache["nc"]


def _host_prepare(x_b, roll, qkv_w, fc_w, fc_b, ln1_g, ln1_b, ln2_g, ln2_b,
                  w1, b1, w2, b2):
    S_, E = x_b.shape
    D = 64
    xr = np.roll(x_b, -roll, axis=0)
    qkvT = np.ascontiguousarray(qkv_w.T).copy()
    qkvT[:, :E] *= D ** -0.5
    return {
        "xT": np.ascontiguousarray(xr.T),
        "qkvT": qkvT,
        "fcT": np.ascontiguousarray(fc_w.T),
        "w1T": np.ascontiguousarray(w1.T),
        "w2T": np.ascontiguousarray(w2.T),
        "ln1": np.stack([ln1_g, ln1_b]).astype(np.float32),
        "ln2": np.stack([ln2_g, ln2_b]).astype(np.float32),
        "fcb": fc_b, "b1": b1, "b2": b2,
        "ones": np.ones((max(512, S_),), np.float32),
    }


def kernel(x, qkv_w, fc_w, fc_b, ln1_g, ln1_b, ln2_g, ln2_b, w1, b1, w2, b2):
    from concourse.bass_utils import run_bass_kernel_spmd

    x = np.ascontiguousarray(np.asarray(x, dtype=np.float32))
    args = [np.ascontiguousarray(np.asarray(a, dtype=np.float32)) for a in
            (qkv_w, fc_w, fc_b, ln1_g, ln1_b, ln2_g, ln2_b, w1, b1, w2, b2)]
    nc = _get_nc()
    in_maps = []
    for c in range(8):
        b, hf = c // 2, c % 2
        in_maps.append(_host_prepare(x[b], hf * (S // 2), *args))
    res = run_bass_kernel_spmd(nc, in_maps, list(range(8)))
    out = np.empty((B, S, E_FULL), np.float32)
    for c in range(8):
        b, hf = c // 2, c % 2
        out[b, hf * (S // 2):(hf + 1) * (S // 2), :] = res.results[c]["out"].T
    return out



# revision 2
# speedup vs baseline: 40.5446x; 40.5446x over previous
"""Trainium2 Bass kernel v2 for a dense transformer block, sharded over 8 NeuronCores.

Sharding: core c handles batch b=c//2 and half hf=c%2 of that batch's 2048
tokens ("own" tokens, rolled to the front). K/V are computed for the full
2048-token batch on both cores of a pair, so no collectives are needed.

v2 vs baseline: bf16 matmul operands everywhere (weights pre-cast host-side),
h/Q/K/V resident in SBUF (no DRAM round-trip), single-pass attention over all
16 heads, 1024-wide exp reads spanning 2 PSUM banks, MLP weights streamed once
(w2 twice), large DMAs only.
"""

import numpy as np

from contextlib import ExitStack

import concourse.bass as bass
import concourse.bacc as bacc
import concourse.tile as tile
import concourse.mybir as mybir

F32 = mybir.dt.float32
F32R = mybir.dt.float32r
BF16 = mybir.dt.bfloat16
AF = mybir.ActivationFunctionType
OP = mybir.AluOpType

EPS = 1e-5


class Cfg:
    def __init__(self, E=1024, H=16, MLP=4096, T_OWN=1024, T_FULL=2048, repeat=1,
                 act="gelu"):
        self.E, self.H, self.MLP = E, H, MLP
        self.T_OWN, self.T_FULL = T_OWN, T_FULL
        self.D = 64
        assert E == self.D * H
        self.NE = E // 128          # 8 feature tiles
        self.NM = MLP // 128        # 32 hidden tiles
        self.NQB = T_OWN // 512     # 2 own-token blocks
        self.NFB = T_FULL // 512    # 4 full-token blocks
        self.NTK = T_FULL // 128    # 16 key tiles
        self.NHP = H // 2           # 8 head pairs (= k/q tile count)
        self.repeat = repeat
        self.act = act


def build(cfg: Cfg):
    E, MLP, T_OWN, T_FULL = cfg.E, cfg.MLP, cfg.T_OWN, cfg.T_FULL

    nc = bacc.Bacc("TRN2", target_bir_lowering=False, debug=False)

    d = {}
    d["xT"] = nc.dram_tensor("xT", [E, T_FULL], F32, kind="ExternalInput")
    d["qkvT"] = nc.dram_tensor("qkvT", [E, 3 * E], BF16, kind="ExternalInput")
    d["fcT"] = nc.dram_tensor("fcT", [E, E], BF16, kind="ExternalInput")
    d["w1T"] = nc.dram_tensor("w1T", [E, MLP], BF16, kind="ExternalInput")
    d["w2T"] = nc.dram_tensor("w2T", [MLP, E], BF16, kind="ExternalInput")
    d["ln1"] = nc.dram_tensor("ln1", [2, E], F32, kind="ExternalInput")
    d["ln2"] = nc.dram_tensor("ln2", [2, E], F32, kind="ExternalInput")
    d["fcb"] = nc.dram_tensor("fcb", [E], F32, kind="ExternalInput")
    d["b1"] = nc.dram_tensor("b1", [MLP], F32, kind="ExternalInput")
    d["b2"] = nc.dram_tensor("b2", [E], F32, kind="ExternalInput")
    d["ones"] = nc.dram_tensor("ones", [T_FULL], F32, kind="ExternalInput")
    d["out"] = nc.dram_tensor("out", [E, T_OWN], F32, kind="ExternalOutput")

    with tile.TileContext(nc) as tc, nc.allow_low_precision(
        reason="bf16 matmul operands by design"
    ):
        if cfg.repeat == 1:
            _body(nc, tc, cfg, d)
        else:
            with tc.For_i(0, cfg.repeat, 1):
                _body(nc, tc, cfg, d)
    nc.compile()
    return nc


def _ln_block_stats(nc, cfg, st_ps, sq_pool, row_pool, stat_pool, srcs,
                    ones_col, eps_t, ones_dram, ones_col_x=None):
    """Column stats over the feature dim for one 512-token block.

    srcs[e] -> [128,512] f32r AP. Returns (srst [1,512] f32r, snb [2,512]
    f32r) tiles: rstd row, and [-mu*rstd ; ones] rows."""
    E, NE = cfg.E, cfg.NE
    s1 = st_ps.tile([1, 512], F32, tag="s1")
    s2 = st_ps.tile([1, 512], F32, tag="s2")
    for e in range(NE):
        src = srcs[e]
        sq = sq_pool.tile([128, 512], F32R, tag="sq")
        if src.dtype == BF16:
            nc.vector.tensor_tensor(sq[:], src, src, OP.mult)
            nc.tensor.matmul(s1[:], ones_col_x[:], src, start=(e == 0),
                             stop=(e == NE - 1))
        else:
            nc.vector.tensor_tensor(sq[:], src.bitcast(F32), src.bitcast(F32),
                                    OP.mult)
            nc.tensor.matmul(s1[:], ones_col[:], src, start=(e == 0),
                             stop=(e == NE - 1))
        nc.tensor.matmul(s2[:], ones_col[:], sq[:], start=(e == 0),
                         stop=(e == NE - 1))
    m_row = row_pool.tile([1, 512], F32, tag="mrow")
    nc.vector.tensor_scalar_mul(m_row[:], s1[:], 1.0 / E)
    v_row = row_pool.tile([1, 512], F32, tag="vrow")
    nc.vector.tensor_scalar_mul(v_row[:], s2[:], 1.0 / E)
    msq = row_pool.tile([1, 512], F32, tag="sd")
    nc.vector.tensor_tensor(msq[:], m_row[:], m_row[:], OP.mult)
    nc.vector.tensor_tensor(v_row[:], v_row[:], msq[:], OP.subtract)
    sd = row_pool.tile([1, 512], F32, tag="sd")
    nc.scalar.activation(sd[:], v_row[:], AF.Sqrt, bias=eps_t[:], scale=1.0)
    srst = stat_pool.tile([1, 512], F32R, tag="srst")
    snb = stat_pool.tile([2, 512], F32R, tag="snb")
    nc.sync.dma_start(snb[1:2, :], ones_dram.ap()[0:512].unsqueeze(0).bitcast(F32R))
    nc.vector.reciprocal(srst[:], sd[:])
    nc.vector.scalar_tensor_tensor(
        snb[0:1, :], m_row[:], -1.0, srst[:].bitcast(F32),
        op0=OP.mult, op1=OP.mult)
    return srst, snb


def _ln_apply(nc, map_ps, gb_e, srst, snb, src_ap_f32, dst_ap):
    """dst = src * (g x rstd) + (g x (-mu*rstd) + b x 1), [128, 512] block."""
    a_ps = map_ps.tile([128, 512], F32, tag="amap")
    nc.tensor.matmul(a_ps[:], gb_e[0:1, :], srst[0:1, :], start=True, stop=True)
    b_ps = map_ps.tile([128, 512], F32, tag="bmap")
    nc.tensor.matmul(b_ps[:], gb_e[0:2, :], snb[0:2, :], start=True, stop=True)
    nc.vector.tensor_tensor(dst_ap, src_ap_f32, a_ps[:], OP.mult)
    nc.vector.tensor_tensor(dst_ap, dst_ap, b_ps[:], OP.add)


def _body(nc, tc, cfg, d):
    E, H, MLP, D = cfg.E, cfg.H, cfg.MLP, cfg.D
    NE, NM, NQB, NFB, NTK, NHP = (cfg.NE, cfg.NM, cfg.NQB, cfg.NFB, cfg.NTK,
                                  cfg.NHP)
    T_OWN, T_FULL = cfg.T_OWN, cfg.T_FULL
    xT, qkvT, fcT, w1T, w2T = d["xT"], d["qkvT"], d["fcT"], d["w1T"], d["w2T"]
    ln1, ln2, fcb, b1, b2 = d["ln1"], d["ln2"], d["fcb"], d["b1"], d["b2"]
    ones, out = d["ones"], d["out"]

    with ExitStack() as ctx:
        consts = ctx.enter_context(tc.tile_pool(name="consts", bufs=1))

        ones_col = consts.tile([128, 1], F32R)
        nc.sync.dma_start(ones_col[:],
                          ones.ap()[0:128].rearrange("(p c) -> p c", c=1)
                          .bitcast(F32R))
        ones_col_bf = consts.tile([128, 1], BF16)
        nc.vector.tensor_copy(ones_col_bf[:], ones_col[:].bitcast(F32))
        ones64_row = consts.tile([1, 64], F32R)
        nc.sync.dma_start(ones64_row[:], ones.ap()[0:64].unsqueeze(0).bitcast(F32R))

        gb1, gb2, fcb_c, b2_c, b1_c = [], [], [], [], []
        for e in range(NE):
            t1 = consts.tile([2, 128], F32R, tag=f"gb1_{e}")
            nc.sync.dma_start(t1[:], ln1.ap()[:, e * 128:(e + 1) * 128].bitcast(F32R))
            gb1.append(t1)
            t2 = consts.tile([2, 128], F32R, tag=f"gb2_{e}")
            nc.sync.dma_start(t2[:], ln2.ap()[:, e * 128:(e + 1) * 128].bitcast(F32R))
            gb2.append(t2)
            t = consts.tile([128, 1], F32, tag=f"fcb_{e}")
            nc.sync.dma_start(t[:], fcb.ap()[e * 128:(e + 1) * 128]
                              .rearrange("(p c) -> p c", c=1))
            fcb_c.append(t)
            t = consts.tile([128, 1], F32, tag=f"b2c_{e}")
            nc.sync.dma_start(t[:], b2.ap()[e * 128:(e + 1) * 128]
                              .rearrange("(p c) -> p c", c=1))
            b2_c.append(t)
        for m in range(NM):
            t = consts.tile([128, 1], F32, tag=f"b1c_{m}")
            nc.sync.dma_start(t[:], b1.ap()[m * 128:(m + 1) * 128]
                              .rearrange("(p c) -> p c", c=1))
            b1_c.append(t)
        eps_t = consts.tile([1, 1], F32)
        nc.vector.memset(eps_t[:], EPS)

        # attention output lives until fc (phase D)
        av_pool = ctx.enter_context(tc.tile_pool(name="avp", bufs=NHP))
        av_tiles = [av_pool.tile([128, T_OWN], BF16, tag="avt", name="avt")
                    for _ in range(NHP)]

        with ExitStack() as pQKV:  # q/k/v live through phase C
            q_pool = pQKV.enter_context(tc.tile_pool(name="qp", bufs=NHP))
            q_tiles = [q_pool.tile([128, T_OWN], BF16, tag="qt", name="qt")
                       for _ in range(NHP)]
            k_pool = pQKV.enter_context(tc.tile_pool(name="kp", bufs=NHP))
            k_tiles = [k_pool.tile([128, T_FULL], BF16, tag="kt", name="kt")
                       for _ in range(NHP)]
            v_pool = pQKV.enter_context(tc.tile_pool(name="vp", bufs=NTK))
            v_tiles = [v_pool.tile([128, H, 65], BF16, tag="vt", name="vt")
                       for _ in range(NTK)]

            with ExitStack() as pH:  # h lives through phase B
                h_pool = pH.enter_context(tc.tile_pool(name="hp", bufs=NE))
                h_tiles = [h_pool.tile([128, T_FULL], BF16, tag="ht", name="ht")
                           for _ in range(NE)]

                # ============ Phase A: LN1 (x resident bf16) ============
                with ExitStack() as pA:
                    x_pool = pA.enter_context(tc.tile_pool(name="xp", bufs=NE))
                    sq_pool = pA.enter_context(tc.tile_pool(name="sqp", bufs=2))
                    st_ps = pA.enter_context(
                        tc.tile_pool(name="stps", bufs=2, space="PSUM"))
                    row_pool = pA.enter_context(tc.tile_pool(name="rows", bufs=1))
                    map_ps = pA.enter_context(
                        tc.tile_pool(name="mapps", bufs=2, space="PSUM"))
                    stat_pool = pA.enter_context(tc.tile_pool(name="statp",
                                                              bufs=2))
                    xts = []
                    for e in range(NE):
                        t = x_pool.tile([128, T_FULL], BF16, tag="xt", name="xt")
                        nc.gpsimd.dma_start(t[:], xT.ap()[e * 128:(e + 1) * 128, :])
                        xts.append(t)
                    for tb in range(NFB):
                        sl = slice(tb * 512, (tb + 1) * 512)
                        srst, snb = _ln_block_stats(
                            nc, cfg, st_ps, sq_pool, row_pool, stat_pool,
                            [t[:, sl] for t in xts], ones_col, eps_t, ones,
                            ones_col_x=ones_col_bf)
                        for e in range(NE):
                            _ln_apply(nc, map_ps, gb1[e], srst, snb,
                                      xts[e][:, sl], h_tiles[e][:, sl])

                # ============ Phase B: QKV projections ============
                with ExitStack() as pB:
                    w_pool = pB.enter_context(tc.tile_pool(name="wqkv", bufs=12))
                    acc_ps = pB.enter_context(
                        tc.tile_pool(name="qkvps", bufs=8, space="PSUM"))

                    def load_group(grp, eng):
                        wts = []
                        for e in range(NE):
                            wt = w_pool.tile([128, 1024], BF16, tag="wg",
                                             name="wg")
                            dma_eng = nc.sync if e % 2 == 0 else nc.scalar
                            dma_eng.dma_start(
                                wt[:], qkvT.ap()[e * 128:(e + 1) * 128,
                                                 grp * 1024:(grp + 1) * 1024])
                            wts.append(wt)
                        return wts

                    # Q: group 0 (cols 0:1024), own tokens only
                    wts = load_group(0, nc.scalar)
                    for jp in range(4):
                        ps = {(j, tqb): acc_ps.tile([128, 512], F32, tag="acc",
                                                    name="acc")
                              for j in range(2) for tqb in range(NQB)}
                        for e in range(NE):
                            for j in range(2):
                                jj = jp * 2 + j
                                for tqb in range(NQB):
                                    nc.tensor.matmul(
                                        ps[(j, tqb)][:],
                                        wts[e][:, jj * 128:(jj + 1) * 128],
                                        h_tiles[e][:, tqb * 512:(tqb + 1) * 512],
                                        start=(e == 0), stop=(e == NE - 1))
                        for j in range(2):
                            hp = jp * 2 + j
                            for tqb in range(NQB):
                                nc.vector.tensor_copy(
                                    q_tiles[hp][:, tqb * 512:(tqb + 1) * 512],
                                    ps[(j, tqb)][:])

                    # K: group 1 (cols 1024:2048), full tokens; 2 j at a time
                    wts = load_group(1, nc.sync)
                    for jj in range(8):
                        ps = {tb: acc_ps.tile([128, 512], F32, tag="acc",
                                              name="acc")
                              for tb in range(NFB)}
                        for e in range(NE):
                            for tb in range(NFB):
                                nc.tensor.matmul(
                                    ps[tb][:],
                                    wts[e][:, jj * 128:(jj + 1) * 128],
                                    h_tiles[e][:, tb * 512:(tb + 1) * 512],
                                    start=(e == 0), stop=(e == NE - 1))
                        for tb in range(NFB):
                            nc.vector.tensor_copy(
                                k_tiles[jj][:, tb * 512:(tb + 1) * 512],
                                ps[tb][:])

                    # V: group 2 (cols 2048:3072) -> [tok, head, d] layout
                    wts = load_group(2, nc.scalar)
                    for vh in range(2):
                        h0 = vh * 8
                        vsl = slice(vh * 512, (vh + 1) * 512)
                        for tk in range(NTK):
                            ps = acc_ps.tile([128, 512], F32, tag="acc",
                                             name="acc")
                            for e in range(NE):
                                nc.tensor.matmul(
                                    ps[:], h_tiles[e][:, tk * 128:(tk + 1) * 128],
                                    wts[e][:, vsl], start=(e == 0),
                                    stop=(e == NE - 1))
                            nc.vector.tensor_copy(
                                v_tiles[tk][:, h0:h0 + 8, 0:64],
                                ps[:].rearrange("p (h d) -> p h d", d=64))
                            if vh == 0:
                                nc.vector.memset(v_tiles[tk][:, :, 64:65], 1.0)

            # ============ Phase C: attention (all 16 heads) ============
            with ExitStack() as pC:
                sc_ps = pC.enter_context(
                    tc.tile_pool(name="scps", bufs=2, space="PSUM"))
                av_ps = pC.enter_context(
                    tc.tile_pool(name="avps", bufs=2, space="PSUM"))
                ex_pool = pC.enter_context(tc.tile_pool(name="exp", bufs=6))
                rr_pool = pC.enter_context(tc.tile_pool(name="rrp", bufs=4))

                for hp in range(NHP):
                    for head in range(2):
                        hd = slice(head * 64, (head + 1) * 64)
                        hg = hp * 2 + head
                        av = av_ps.tile([65, T_OWN], F32, tag="av", name="av")
                        for tk in range(NTK):
                            ksl = slice(tk * 128, (tk + 1) * 128)
                            sc = sc_ps.tile([128, T_OWN], F32, tag="sc", name="sc")
                            for tqb in range(NQB):
                                qsl = slice(tqb * 512, (tqb + 1) * 512)
                                nc.tensor.matmul(sc[:, qsl], k_tiles[hp][hd, ksl],
                                                 q_tiles[hp][hd, qsl],
                                                 start=True, stop=True)
                            ex = ex_pool.tile([128, T_OWN], BF16, tag="ex",
                                              name="ex")
                            nc.scalar.activation(ex[:], sc[:], AF.Exp)
                            for tqb in range(NQB):
                                qsl = slice(tqb * 512, (tqb + 1) * 512)
                                nc.tensor.matmul(av[:, qsl],
                                                 v_tiles[tk][:, hg, :],
                                                 ex[:, qsl], start=(tk == 0),
                                                 stop=(tk == NTK - 1))
                        # normalize: av[0:64] * broadcast(1/av[64])
                        rrow = rr_pool.tile([1, T_OWN], F32R, tag="rr", name="rr")
                        nc.vector.reciprocal(rrow[:], av[64:65, :])
                        rm = sc_ps.tile([64, T_OWN], F32, tag="sc", name="rm")
                        for tqb in range(NQB):
                            qsl = slice(tqb * 512, (tqb + 1) * 512)
                            nc.tensor.matmul(rm[:, qsl], ones64_row[:],
                                             rrow[0:1, qsl].bitcast(F32R),
                                             start=True, stop=True)
                        rms = rr_pool.tile([64, T_OWN], F32, tag="rms", name="rms")
                        nc.vector.tensor_copy(rms[:], rm[:])
                        nc.vector.tensor_tensor(av_tiles[hp][hd, :], av[0:64, :],
                                                rms[:], OP.mult)

        # ============ Phase D: fc_out + residual ============
        x2_pool = ctx.enter_context(tc.tile_pool(name="x2p", bufs=NE))
        x2_tiles = [x2_pool.tile([128, T_OWN], F32R, tag="x2t", name="x2t")
                    for _ in range(NE)]
        with ExitStack() as pD:
            wf_pool = pD.enter_context(tc.tile_pool(name="wfp", bufs=NE))
            xr_pool = pD.enter_context(tc.tile_pool(name="xrp", bufs=3))
            fc_ps = pD.enter_context(tc.tile_pool(name="fcps", bufs=8, space="PSUM"))
            wf = []
            for e in range(NE):
                wt = wf_pool.tile([128, E], BF16, tag="wf", name="wf")
                dma_eng = nc.sync if e % 2 == 0 else nc.scalar
                dma_eng.dma_start(wt[:], fcT.ap()[e * 128:(e + 1) * 128, :])
                wf.append(wt)
            for oh in range(4):
                ps = {(o, tqb): fc_ps.tile([128, 512], F32, tag="fc", name="fc")
                      for o in range(2) for tqb in range(NQB)}
                for e in range(NE):
                    for o in range(2):
                        oo = oh * 2 + o
                        for tqb in range(NQB):
                            nc.tensor.matmul(
                                ps[(o, tqb)][:],
                                wf[e][:, oo * 128:(oo + 1) * 128],
                                av_tiles[e][:, tqb * 512:(tqb + 1) * 512],
                                start=(e == 0), stop=(e == NE - 1))
                for o in range(2):
                    oo = oh * 2 + o
                    xr = xr_pool.tile([128, T_OWN], F32, tag="xr")
                    nc.sync.dma_start(xr[:],
                                      xT.ap()[oo * 128:(oo + 1) * 128, 0:T_OWN])
                    for tqb in range(NQB):
                        sl = slice(tqb * 512, (tqb + 1) * 512)
                        nc.vector.scalar_tensor_tensor(
                            x2_tiles[oo][:, sl], ps[(o, tqb)][:], fcb_c[oo][:],
                            xr[:, sl], op0=OP.add, op1=OP.add)

        # ============ Phase E: LN2 ============
        h2_pool = ctx.enter_context(tc.tile_pool(name="h2p", bufs=NE))
        h2_tiles = [h2_pool.tile([128, T_OWN], BF16, tag="h2t", name="h2t")
                    for _ in range(NE)]
        with ExitStack() as pE:
            sq_pool = pE.enter_context(tc.tile_pool(name="sq2p", bufs=2))
            st_ps = pE.enter_context(
                tc.tile_pool(name="st2ps", bufs=2, space="PSUM"))
            row_pool = pE.enter_context(tc.tile_pool(name="rows2", bufs=2))
            map_ps = pE.enter_context(
                tc.tile_pool(name="map2ps", bufs=2, space="PSUM"))
            stat2_pool = pE.enter_context(tc.tile_pool(name="stat2p", bufs=2))
            for tb in range(NQB):
                sl = slice(tb * 512, (tb + 1) * 512)
                srst, snb = _ln_block_stats(
                    nc, cfg, st_ps, sq_pool, row_pool, stat2_pool,
                    [x2_tiles[e][:, sl] for e in range(NE)],
                    ones_col, eps_t, ones)
                for e in range(NE):
                    _ln_apply(nc, map_ps, gb2[e], srst, snb,
                              x2_tiles[e][:, sl].bitcast(F32), h2_tiles[e][:, sl])

        # ============ Phase F: MLP ============
        g_pool = ctx.enter_context(tc.tile_pool(name="gp", bufs=NM))
        g_tiles = [g_pool.tile([128, T_OWN], BF16, tag="gt", name="gt")
                   for _ in range(NM)]
        with ExitStack() as pF1:
            w1_pool = pF1.enter_context(tc.tile_pool(name="w1p", bufs=10))
            m1_ps = pF1.enter_context(
                tc.tile_pool(name="m1ps", bufs=4, space="PSUM"))
            for half in range(2):
                w1h = []
                for e in range(NE):
                    wt = w1_pool.tile([128, MLP // 2], BF16, tag="w1")
                    dma_eng = nc.sync if e % 2 == 0 else nc.scalar
                    dma_eng.dma_start(
                        wt[:], w1T.ap()[e * 128:(e + 1) * 128,
                                        half * (MLP // 2):(half + 1) * (MLP // 2)])
                    w1h.append(wt)
                for mbl in range(NM // 8):
                    mb = half * (NM // 8) + mbl
                    for jp in range(2):
                        ps = {j: m1_ps.tile([128, T_OWN], F32, tag="m1",
                                            name="m1")
                              for j in range(2)}
                        for e in range(NE):
                            for j in range(2):
                                jj = jp * 2 + j
                                csl = slice(mbl * 512 + jj * 128,
                                            mbl * 512 + (jj + 1) * 128)
                                for tqb in range(NQB):
                                    nc.tensor.matmul(
                                        ps[j][:, tqb * 512:(tqb + 1) * 512],
                                        w1h[e][:, csl],
                                        h2_tiles[e][:, tqb * 512:(tqb + 1) * 512],
                                        start=(e == 0), stop=(e == NE - 1))
                        for j in range(2):
                            m = mb * 4 + jp * 2 + j
                            nc.scalar.activation(g_tiles[m][:], ps[j][:],
                                                 AF.Gelu if cfg.act == "gelu"
                                                 else AF.Tanh,
                                                 bias=b1_c[m][:], scale=1.0)

        with ExitStack() as pF2:
            w2_pool = pF2.enter_context(tc.tile_pool(name="w2p", bufs=8))
            out_pool = pF2.enter_context(tc.tile_pool(name="op", bufs=4))
            m2_ps = pF2.enter_context(
                tc.tile_pool(name="m2ps", bufs=NE, space="PSUM"))
            for oh in range(2):
                osl_w = slice(oh * 512, (oh + 1) * 512)
                ps = {(o, tqb): m2_ps.tile([128, 512], F32, tag="m2", name="m2")
                      for o in range(4) for tqb in range(NQB)}
                for m in range(NM):
                    wt = w2_pool.tile([128, 512], BF16, tag="w2")
                    dma_eng = nc.sync if m % 2 == 0 else nc.scalar
                    dma_eng.dma_start(
                        wt[:], w2T.ap()[m * 128:(m + 1) * 128, osl_w])
                    for o in range(4):
                        for tqb in range(NQB):
                            nc.tensor.matmul(
                                ps[(o, tqb)][:], wt[:, o * 128:(o + 1) * 128],
                                g_tiles[m][:, tqb * 512:(tqb + 1) * 512],
                                start=(m == 0), stop=(m == NM - 1))
                for o in range(4):
                    oo = oh * 4 + o
                    ot = out_pool.tile([128, T_OWN], F32, tag="ot", name="ot")
                    for tqb in range(NQB):
                        sl = slice(tqb * 512, (tqb + 1) * 512)
                        nc.vector.scalar_tensor_tensor(
                            ot[:, sl], ps[(o, tqb)][:], b2_c[oo][:],
                            x2_tiles[oo][:, sl].bitcast(F32),
                            op0=OP.add, op1=OP.add)
                    nc.sync.dma_start(out.ap()[oo * 128:(oo + 1) * 128, :], ot[:])


# ----------------------------------------------------------------------------
# host driver
# ----------------------------------------------------------------------------
B, S, E_FULL, H_FULL, MLP_FULL = 4, 2048, 1024, 16, 4096
_cache = {}


def _get_nc():
    if "nc" not in _cache:
        _cache["nc"] = build(Cfg())
    return _cache["nc"]


def _bf16(a):
    import ml_dtypes
    return np.ascontiguousarray(a.astype(ml_dtypes.bfloat16))


def _host_prepare(x_b, roll, qkv_w, fc_w, fc_b, ln1_g, ln1_b, ln2_g, ln2_b,
                  w1, b1, w2, b2):
    S_, E = x_b.shape
    D = 64
    xr = np.roll(x_b, -roll, axis=0)
    qkvT = np.ascontiguousarray(qkv_w.T).copy()
    qkvT[:, :E] *= D ** -0.5
    return {
        "xT": np.ascontiguousarray(xr.T),
        "qkvT": _bf16(qkvT),
        "fcT": _bf16(np.ascontiguousarray(fc_w.T)),
        "w1T": _bf16(np.ascontiguousarray(w1.T)),
        "w2T": _bf16(np.ascontiguousarray(w2.T)),
        "ln1": np.stack([ln1_g, ln1_b]).astype(np.float32),
        "ln2": np.stack([ln2_g, ln2_b]).astype(np.float32),
        "fcb": fc_b, "b1": b1, "b2": b2,
        "ones": np.ones((S_,), np.float32),
    }


def kernel(x, qkv_w, fc_w, fc_b, ln1_g, ln1_b, ln2_g, ln2_b, w1, b1, w2, b2):
    from concourse.bass_utils import run_bass_kernel_spmd

    x = np.ascontiguousarray(np.asarray(x, dtype=np.float32))
    args = [np.ascontiguousarray(np.asarray(a, dtype=np.float32)) for a in
            (qkv_w, fc_w, fc_b, ln1_g, ln1_b, ln2_g, ln2_b, w1, b1, w2, b2)]
    nc = _get_nc()
    in_maps = []
    for c in range(8):
        b, hf = c // 2, c % 2
        in_maps.append(_host_prepare(x[b], hf * (S // 2), *args))
    res = run_bass_kernel_spmd(nc, in_maps, list(range(8)))
    out = np.empty((B, S, E_FULL), np.float32)
    for c in range(8):
        b, hf = c // 2, c % 2
        out[b, hf * (S // 2):(hf + 1) * (S // 2), :] = res.results[c]["out"].T
    return out


# revision 3
# speedup vs baseline: 50.8631x; 1.2545x over previous
"""Trainium2 Bass kernel v2 for a dense transformer block, sharded over 8 NeuronCores.

Sharding: core c handles batch b=c//2 and half hf=c%2 of that batch's 2048
tokens ("own" tokens, rolled to the front). K/V are computed for the full
2048-token batch on both cores of a pair, so no collectives are needed.

v2 vs baseline: bf16 matmul operands everywhere (weights pre-cast host-side),
h/Q/K/V resident in SBUF (no DRAM round-trip), single-pass attention over all
16 heads, 1024-wide exp reads spanning 2 PSUM banks, MLP weights streamed once
(w2 twice), large DMAs only.
"""

import numpy as np

from contextlib import ExitStack

import concourse.bass as bass
import concourse.bacc as bacc
import concourse.tile as tile
import concourse.mybir as mybir

F32 = mybir.dt.float32
F32R = mybir.dt.float32r
BF16 = mybir.dt.bfloat16
AF = mybir.ActivationFunctionType
OP = mybir.AluOpType

EPS = 1e-5


class Cfg:
    def __init__(self, E=1024, H=16, MLP=4096, T_OWN=1024, T_FULL=2048, repeat=1,
                 act="gelu"):
        self.E, self.H, self.MLP = E, H, MLP
        self.T_OWN, self.T_FULL = T_OWN, T_FULL
        self.D = 64
        assert E == self.D * H
        self.NE = E // 128          # 8 feature tiles
        self.NM = MLP // 128        # 32 hidden tiles
        self.NQB = T_OWN // 512     # 2 own-token blocks
        self.NFB = T_FULL // 512    # 4 full-token blocks
        self.NTK = T_FULL // 128    # 16 key tiles
        self.NHP = H // 2           # 8 head pairs (= k/q tile count)
        self.repeat = repeat
        self.act = act


def build(cfg: Cfg):
    E, MLP, T_OWN, T_FULL = cfg.E, cfg.MLP, cfg.T_OWN, cfg.T_FULL

    nc = bacc.Bacc("TRN2", target_bir_lowering=False, debug=False)

    d = {}
    d["xT"] = nc.dram_tensor("xT", [E, T_FULL], F32, kind="ExternalInput")
    d["qkvT"] = nc.dram_tensor("qkvT", [E, 3 * E], BF16, kind="ExternalInput")
    d["fcT"] = nc.dram_tensor("fcT", [E, E], BF16, kind="ExternalInput")
    d["w1T"] = nc.dram_tensor("w1T", [E, MLP], BF16, kind="ExternalInput")
    d["w2T"] = nc.dram_tensor("w2T", [MLP, E], BF16, kind="ExternalInput")
    d["ln1"] = nc.dram_tensor("ln1", [2, E], F32, kind="ExternalInput")
    d["ln2"] = nc.dram_tensor("ln2", [2, E], F32, kind="ExternalInput")
    d["fcb"] = nc.dram_tensor("fcb", [E], F32, kind="ExternalInput")
    d["b1"] = nc.dram_tensor("b1", [MLP], F32, kind="ExternalInput")
    d["b2"] = nc.dram_tensor("b2", [E], F32, kind="ExternalInput")
    d["ones"] = nc.dram_tensor("ones", [T_FULL], F32, kind="ExternalInput")
    d["out"] = nc.dram_tensor("out", [E, T_OWN], F32, kind="ExternalOutput")

    with tile.TileContext(nc) as tc, nc.allow_low_precision(
        reason="bf16 matmul operands by design"
    ):
        if cfg.repeat == 1:
            _body(nc, tc, cfg, d)
        else:
            with tc.For_i(0, cfg.repeat, 1):
                _body(nc, tc, cfg, d)
    nc.compile()
    return nc


def _ln_block_stats(nc, cfg, st_ps, sq_pool, row_pool, stat_pool, srcs,
                    ones_col, eps_t, ones_dram, ones_col_x=None):
    """Column stats over the feature dim for one 512-token block.

    srcs[e] -> [128,512] f32r AP. Returns (srst [1,512] f32r, snb [2,512]
    f32r) tiles: rstd row, and [-mu*rstd ; ones] rows."""
    E, NE = cfg.E, cfg.NE
    s1 = st_ps.tile([1, 512], F32, tag="s1")
    s2 = st_ps.tile([1, 512], F32, tag="s2")
    for e in range(NE):
        src = srcs[e]
        sq = sq_pool.tile([128, 512], F32R, tag="sq")
        if src.dtype == BF16:
            nc.vector.tensor_tensor(sq[:], src, src, OP.mult)
            nc.tensor.matmul(s1[:], ones_col_x[:], src, start=(e == 0),
                             stop=(e == NE - 1))
        else:
            nc.vector.tensor_tensor(sq[:], src.bitcast(F32), src.bitcast(F32),
                                    OP.mult)
            nc.tensor.matmul(s1[:], ones_col[:], src, start=(e == 0),
                             stop=(e == NE - 1))
        nc.tensor.matmul(s2[:], ones_col[:], sq[:], start=(e == 0),
                         stop=(e == NE - 1))
    m_row = row_pool.tile([1, 512], F32, tag="mrow")
    nc.vector.tensor_scalar_mul(m_row[:], s1[:], 1.0 / E)
    v_row = row_pool.tile([1, 512], F32, tag="vrow")
    nc.vector.tensor_scalar_mul(v_row[:], s2[:], 1.0 / E)
    msq = row_pool.tile([1, 512], F32, tag="sd")
    nc.vector.tensor_tensor(msq[:], m_row[:], m_row[:], OP.mult)
    nc.vector.tensor_tensor(v_row[:], v_row[:], msq[:], OP.subtract)
    sd = row_pool.tile([1, 512], F32, tag="sd")
    nc.scalar.activation(sd[:], v_row[:], AF.Sqrt, bias=eps_t[:], scale=1.0)
    srst = stat_pool.tile([1, 512], F32R, tag="srst")
    snb = stat_pool.tile([2, 512], F32R, tag="snb")
    nc.sync.dma_start(snb[1:2, :], ones_dram.ap()[0:512].unsqueeze(0).bitcast(F32R))
    nc.vector.reciprocal(srst[:], sd[:])
    nc.vector.scalar_tensor_tensor(
        snb[0:1, :], m_row[:], -1.0, srst[:].bitcast(F32),
        op0=OP.mult, op1=OP.mult)
    return srst, snb


def _ln_apply(nc, map_ps, gb_e, srst, snb, src_ap_f32, dst_ap):
    """dst = src * (g x rstd) + (g x (-mu*rstd) + b x 1), [128, 512] block."""
    a_ps = map_ps.tile([128, 512], F32, tag="amap")
    nc.tensor.matmul(a_ps[:], gb_e[0:1, :], srst[0:1, :], start=True, stop=True)
    b_ps = map_ps.tile([128, 512], F32, tag="bmap")
    nc.tensor.matmul(b_ps[:], gb_e[0:2, :], snb[0:2, :], start=True, stop=True)
    nc.vector.tensor_tensor(dst_ap, src_ap_f32, a_ps[:], OP.mult)
    nc.vector.tensor_tensor(dst_ap, dst_ap, b_ps[:], OP.add)


def _body(nc, tc, cfg, d):
    E, H, MLP, D = cfg.E, cfg.H, cfg.MLP, cfg.D
    NE, NM, NQB, NFB, NTK, NHP = (cfg.NE, cfg.NM, cfg.NQB, cfg.NFB, cfg.NTK,
                                  cfg.NHP)
    T_OWN, T_FULL = cfg.T_OWN, cfg.T_FULL
    xT, qkvT, fcT, w1T, w2T = d["xT"], d["qkvT"], d["fcT"], d["w1T"], d["w2T"]
    ln1, ln2, fcb, b1, b2 = d["ln1"], d["ln2"], d["fcb"], d["b1"], d["b2"]
    ones, out = d["ones"], d["out"]

    with ExitStack() as ctx:
        consts = ctx.enter_context(tc.tile_pool(name="consts", bufs=1))

        ones_col = consts.tile([128, 1], F32R)
        nc.sync.dma_start(ones_col[:],
                          ones.ap()[0:128].rearrange("(p c) -> p c", c=1)
                          .bitcast(F32R))
        ones_col_bf = consts.tile([128, 1], BF16)
        nc.vector.tensor_copy(ones_col_bf[:], ones_col[:].bitcast(F32))
        ones64_row = consts.tile([1, 64], F32R)
        nc.sync.dma_start(ones64_row[:], ones.ap()[0:64].unsqueeze(0).bitcast(F32R))

        gb1, gb2, fcb_c, b2_c, b1_c = [], [], [], [], []
        for e in range(NE):
            t1 = consts.tile([2, 128], F32R, tag=f"gb1_{e}")
            nc.sync.dma_start(t1[:], ln1.ap()[:, e * 128:(e + 1) * 128].bitcast(F32R))
            gb1.append(t1)
            t2 = consts.tile([2, 128], F32R, tag=f"gb2_{e}")
            nc.sync.dma_start(t2[:], ln2.ap()[:, e * 128:(e + 1) * 128].bitcast(F32R))
            gb2.append(t2)
            t = consts.tile([128, 1], F32, tag=f"fcb_{e}")
            nc.sync.dma_start(t[:], fcb.ap()[e * 128:(e + 1) * 128]
                              .rearrange("(p c) -> p c", c=1))
            fcb_c.append(t)
            t = consts.tile([128, 1], F32, tag=f"b2c_{e}")
            nc.sync.dma_start(t[:], b2.ap()[e * 128:(e + 1) * 128]
                              .rearrange("(p c) -> p c", c=1))
            b2_c.append(t)
        for m in range(NM):
            t = consts.tile([128, 1], F32, tag=f"b1c_{m}")
            nc.sync.dma_start(t[:], b1.ap()[m * 128:(m + 1) * 128]
                              .rearrange("(p c) -> p c", c=1))
            b1_c.append(t)
        eps_t = consts.tile([1, 1], F32)
        nc.vector.memset(eps_t[:], EPS)

        # attention output lives until fc (phase D)
        av_pool = ctx.enter_context(tc.tile_pool(name="avp", bufs=NHP))
        av_tiles = [av_pool.tile([128, T_OWN], BF16, tag="avt", name="avt")
                    for _ in range(NHP)]

        with ExitStack() as pQKV:  # q/k/v live through phase C
            q_pool = pQKV.enter_context(tc.tile_pool(name="qp", bufs=NHP))
            q_tiles = [q_pool.tile([128, T_OWN], BF16, tag="qt", name="qt")
                       for _ in range(NHP)]
            k_pool = pQKV.enter_context(tc.tile_pool(name="kp", bufs=NHP))
            k_tiles = [k_pool.tile([128, T_FULL], BF16, tag="kt", name="kt")
                       for _ in range(NHP)]
            v_pool = pQKV.enter_context(tc.tile_pool(name="vp", bufs=NTK))
            v_tiles = [v_pool.tile([128, H, 65], BF16, tag="vt", name="vt")
                       for _ in range(NTK)]

            with ExitStack() as pH:  # h lives through phase B
                h_pool = pH.enter_context(tc.tile_pool(name="hp", bufs=NE))
                h_tiles = [h_pool.tile([128, T_FULL], BF16, tag="ht", name="ht")
                           for _ in range(NE)]

                # ============ Phase A: LN1 (x resident bf16) ============
                with ExitStack() as pA:
                    x_pool = pA.enter_context(tc.tile_pool(name="xp", bufs=NE))
                    sq_pool = pA.enter_context(tc.tile_pool(name="sqp", bufs=2))
                    st_ps = pA.enter_context(
                        tc.tile_pool(name="stps", bufs=2, space="PSUM"))
                    row_pool = pA.enter_context(tc.tile_pool(name="rows", bufs=1))
                    map_ps = pA.enter_context(
                        tc.tile_pool(name="mapps", bufs=2, space="PSUM"))
                    stat_pool = pA.enter_context(tc.tile_pool(name="statp",
                                                              bufs=2))
                    xts = []
                    for e in range(NE):
                        t = x_pool.tile([128, T_FULL], BF16, tag="xt", name="xt")
                        nc.gpsimd.dma_start(t[:], xT.ap()[e * 128:(e + 1) * 128, :])
                        xts.append(t)
                    for tb in range(NFB):
                        sl = slice(tb * 512, (tb + 1) * 512)
                        srst, snb = _ln_block_stats(
                            nc, cfg, st_ps, sq_pool, row_pool, stat_pool,
                            [t[:, sl] for t in xts], ones_col, eps_t, ones,
                            ones_col_x=ones_col_bf)
                        for e in range(NE):
                            _ln_apply(nc, map_ps, gb1[e], srst, snb,
                                      xts[e][:, sl], h_tiles[e][:, sl])

                # ===== Phases B+C interleaved: V first, then per-hp =====
                # K/Q projections followed by that pair's attention, so the
                # scheduler fills exp (ACT) stalls with the next pair's MMs.
                with ExitStack() as pBC:
                    w_pool = pBC.enter_context(tc.tile_pool(name="wqkv",
                                                            bufs=16))
                    acc_ps = pBC.enter_context(
                        tc.tile_pool(name="qkvps", bufs=2, space="PSUM"))
                    sc_ps = pBC.enter_context(
                        tc.tile_pool(name="scps", bufs=2, space="PSUM"))
                    av_ps = pBC.enter_context(
                        tc.tile_pool(name="avps", bufs=2, space="PSUM"))
                    ex_pool = pBC.enter_context(tc.tile_pool(name="exp",
                                                             bufs=4))
                    rr_pool = pBC.enter_context(tc.tile_pool(name="rrp",
                                                             bufs=2))

                    def load_group(grp):
                        wts = []
                        for e in range(NE):
                            wt = w_pool.tile([128, 1024], BF16, tag="wg",
                                             name="wg")
                            dma_eng = nc.sync if e % 2 == 0 else nc.scalar
                            dma_eng.dma_start(
                                wt[:], qkvT.ap()[e * 128:(e + 1) * 128,
                                                 grp * 1024:(grp + 1) * 1024])
                            wts.append(wt)
                        return wts

                    # V (cols 2048:3072) -> [tok, head, d] layout
                    wv = load_group(2)
                    wk = load_group(1)
                    for vh in range(2):
                        h0 = vh * 8
                        vsl = slice(vh * 512, (vh + 1) * 512)
                        for tk in range(NTK):
                            ps = acc_ps.tile([128, 512], F32, tag="acc",
                                             name="acc")
                            for e in range(NE):
                                nc.tensor.matmul(
                                    ps[:],
                                    h_tiles[e][:, tk * 128:(tk + 1) * 128],
                                    wv[e][:, vsl], start=(e == 0),
                                    stop=(e == NE - 1))
                            nc.vector.tensor_copy(
                                v_tiles[tk][:, h0:h0 + 8, 0:64],
                                ps[:].rearrange("p (h d) -> p h d", d=64))
                            if vh == 0:
                                nc.vector.memset(v_tiles[tk][:, :, 64:65], 1.0)
                    wq = load_group(0)

                    for hp in range(NHP):
                        # K projection for this head pair (col jj of k-group)
                        for tp in range(2):
                            ps = {tb: acc_ps.tile([128, 512], F32, tag="acc",
                                                  name="acc")
                                  for tb in range(2)}
                            for e in range(NE):
                                for tb in range(2):
                                    tbb = tp * 2 + tb
                                    nc.tensor.matmul(
                                        ps[tb][:],
                                        wk[e][:, hp * 128:(hp + 1) * 128],
                                        h_tiles[e][:, tbb * 512:(tbb + 1) * 512],
                                        start=(e == 0), stop=(e == NE - 1))
                            for tb in range(2):
                                tbb = tp * 2 + tb
                                nc.vector.tensor_copy(
                                    k_tiles[hp][:, tbb * 512:(tbb + 1) * 512],
                                    ps[tb][:])
                        # Q projection for this head pair
                        ps = {tqb: acc_ps.tile([128, 512], F32, tag="acc",
                                               name="acc")
                              for tqb in range(NQB)}
                        for e in range(NE):
                            for tqb in range(NQB):
                                nc.tensor.matmul(
                                    ps[tqb][:],
                                    wq[e][:, hp * 128:(hp + 1) * 128],
                                    h_tiles[e][:, tqb * 512:(tqb + 1) * 512],
                                    start=(e == 0), stop=(e == NE - 1))
                        for tqb in range(NQB):
                            nc.vector.tensor_copy(
                                q_tiles[hp][:, tqb * 512:(tqb + 1) * 512],
                                ps[tqb][:])

                        # attention for this head pair
                        for tqb in range(NQB):
                            qsl = slice(tqb * 512, (tqb + 1) * 512)
                            avs = [av_ps.tile([65, 512], F32, tag="av",
                                              name="av") for _ in range(2)]
                            for tk in range(NTK):
                                ksl = slice(tk * 128, (tk + 1) * 128)
                                sc = sc_ps.tile([128, 1024], F32, tag="sc",
                                                name="sc")
                                for head in range(2):
                                    hd = slice(head * 64, (head + 1) * 64)
                                    nc.tensor.matmul(
                                        sc[:, head * 512:(head + 1) * 512],
                                        k_tiles[hp][hd, ksl],
                                        q_tiles[hp][hd, qsl],
                                        start=True, stop=True)
                                ex = ex_pool.tile([128, 1024], BF16, tag="ex",
                                                  name="ex")
                                nc.scalar.activation(ex[:], sc[:], AF.Exp)
                                for head in range(2):
                                    hg = hp * 2 + head
                                    nc.tensor.matmul(
                                        avs[head][:],
                                        v_tiles[tk][:, hg, :],
                                        ex[:, head * 512:(head + 1) * 512],
                                        start=(tk == 0), stop=(tk == NTK - 1))
                            for head in range(2):
                                hd = slice(head * 64, (head + 1) * 64)
                                av = avs[head]
                                rrow = rr_pool.tile([1, 512], F32R, tag="rr",
                                                    name="rr")
                                nc.vector.reciprocal(rrow[:], av[64:65, :])
                                rm = sc_ps.tile([64, 512], F32, tag="sc",
                                                name="rm")
                                nc.tensor.matmul(rm[:], ones64_row[:],
                                                 rrow[0:1, :].bitcast(F32R),
                                                 start=True, stop=True)
                                rms = rr_pool.tile([64, 512], F32, tag="rms",
                                                   name="rms")
                                nc.vector.tensor_copy(rms[:], rm[:])
                                nc.vector.tensor_tensor(
                                    av_tiles[hp][hd, qsl], av[0:64, :],
                                    rms[:], OP.mult)

        # ============ Phase D: fc_out + residual ============
        x2_pool = ctx.enter_context(tc.tile_pool(name="x2p", bufs=NE))
        x2_tiles = [x2_pool.tile([128, T_OWN], F32R, tag="x2t", name="x2t")
                    for _ in range(NE)]
        with ExitStack() as pD:
            wf_pool = pD.enter_context(tc.tile_pool(name="wfp", bufs=NE))
            xr_pool = pD.enter_context(tc.tile_pool(name="xrp", bufs=3))
            fc_ps = pD.enter_context(tc.tile_pool(name="fcps", bufs=8, space="PSUM"))
            wf = []
            for e in range(NE):
                wt = wf_pool.tile([128, E], BF16, tag="wf", name="wf")
                dma_eng = nc.sync if e % 2 == 0 else nc.scalar
                dma_eng.dma_start(wt[:], fcT.ap()[e * 128:(e + 1) * 128, :])
                wf.append(wt)
            for oh in range(4):
                ps = {(o, tqb): fc_ps.tile([128, 512], F32, tag="fc", name="fc")
                      for o in range(2) for tqb in range(NQB)}
                for e in range(NE):
                    for o in range(2):
                        oo = oh * 2 + o
                        for tqb in range(NQB):
                            nc.tensor.matmul(
                                ps[(o, tqb)][:],
                                wf[e][:, oo * 128:(oo + 1) * 128],
                                av_tiles[e][:, tqb * 512:(tqb + 1) * 512],
                                start=(e == 0), stop=(e == NE - 1))
                for o in range(2):
                    oo = oh * 2 + o
                    xr = xr_pool.tile([128, T_OWN], F32, tag="xr")
                    nc.sync.dma_start(xr[:],
                                      xT.ap()[oo * 128:(oo + 1) * 128, 0:T_OWN])
                    for tqb in range(NQB):
                        sl = slice(tqb * 512, (tqb + 1) * 512)
                        nc.vector.scalar_tensor_tensor(
                            x2_tiles[oo][:, sl], ps[(o, tqb)][:], fcb_c[oo][:],
                            xr[:, sl], op0=OP.add, op1=OP.add)

        # ============ Phase E: LN2 ============
        h2_pool = ctx.enter_context(tc.tile_pool(name="h2p", bufs=NE))
        h2_tiles = [h2_pool.tile([128, T_OWN], BF16, tag="h2t", name="h2t")
                    for _ in range(NE)]
        with ExitStack() as pE:
            sq_pool = pE.enter_context(tc.tile_pool(name="sq2p", bufs=2))
            st_ps = pE.enter_context(
                tc.tile_pool(name="st2ps", bufs=2, space="PSUM"))
            row_pool = pE.enter_context(tc.tile_pool(name="rows2", bufs=2))
            map_ps = pE.enter_context(
                tc.tile_pool(name="map2ps", bufs=2, space="PSUM"))
            stat2_pool = pE.enter_context(tc.tile_pool(name="stat2p", bufs=2))
            for tb in range(NQB):
                sl = slice(tb * 512, (tb + 1) * 512)
                srst, snb = _ln_block_stats(
                    nc, cfg, st_ps, sq_pool, row_pool, stat2_pool,
                    [x2_tiles[e][:, sl] for e in range(NE)],
                    ones_col, eps_t, ones)
                for e in range(NE):
                    _ln_apply(nc, map_ps, gb2[e], srst, snb,
                              x2_tiles[e][:, sl].bitcast(F32), h2_tiles[e][:, sl])

        # ============ Phase F: MLP ============
        g_pool = ctx.enter_context(tc.tile_pool(name="gp", bufs=NM))
        g_tiles = [g_pool.tile([128, T_OWN], BF16, tag="gt", name="gt")
                   for _ in range(NM)]
        with ExitStack() as pF1:
            w1_pool = pF1.enter_context(tc.tile_pool(name="w1p", bufs=10))
            m1_ps = pF1.enter_context(
                tc.tile_pool(name="m1ps", bufs=4, space="PSUM"))
            for half in range(2):
                w1h = []
                for e in range(NE):
                    wt = w1_pool.tile([128, MLP // 2], BF16, tag="w1")
                    dma_eng = nc.sync if e % 2 == 0 else nc.scalar
                    dma_eng.dma_start(
                        wt[:], w1T.ap()[e * 128:(e + 1) * 128,
                                        half * (MLP // 2):(half + 1) * (MLP // 2)])
                    w1h.append(wt)
                for mbl in range(NM // 8):
                    mb = half * (NM // 8) + mbl
                    for jp in range(2):
                        ps = {j: m1_ps.tile([128, T_OWN], F32, tag="m1",
                                            name="m1")
                              for j in range(2)}
                        for e in range(NE):
                            for j in range(2):
                                jj = jp * 2 + j
                                csl = slice(mbl * 512 + jj * 128,
                                            mbl * 512 + (jj + 1) * 128)
                                for tqb in range(NQB):
                                    nc.tensor.matmul(
                                        ps[j][:, tqb * 512:(tqb + 1) * 512],
                                        w1h[e][:, csl],
                                        h2_tiles[e][:, tqb * 512:(tqb + 1) * 512],
                                        start=(e == 0), stop=(e == NE - 1))
                        for j in range(2):
                            m = mb * 4 + jp * 2 + j
                            nc.scalar.activation(g_tiles[m][:], ps[j][:],
                                                 AF.Gelu if cfg.act == "gelu"
                                                 else AF.Tanh,
                                                 bias=b1_c[m][:], scale=1.0)

        with ExitStack() as pF2:
            w2_pool = pF2.enter_context(tc.tile_pool(name="w2p", bufs=8))
            out_pool = pF2.enter_context(tc.tile_pool(name="op", bufs=4))
            m2_ps = pF2.enter_context(
                tc.tile_pool(name="m2ps", bufs=NE, space="PSUM"))
            for oh in range(2):
                osl_w = slice(oh * 512, (oh + 1) * 512)
                ps = {(o, tqb): m2_ps.tile([128, 512], F32, tag="m2", name="m2")
                      for o in range(4) for tqb in range(NQB)}
                for m in range(NM):
                    wt = w2_pool.tile([128, 512], BF16, tag="w2")
                    dma_eng = nc.sync if m % 2 == 0 else nc.scalar
                    dma_eng.dma_start(
                        wt[:], w2T.ap()[m * 128:(m + 1) * 128, osl_w])
                    for o in range(4):
                        for tqb in range(NQB):
                            nc.tensor.matmul(
                                ps[(o, tqb)][:], wt[:, o * 128:(o + 1) * 128],
                                g_tiles[m][:, tqb * 512:(tqb + 1) * 512],
                                start=(m == 0), stop=(m == NM - 1))
                for o in range(4):
                    oo = oh * 4 + o
                    ot = out_pool.tile([128, T_OWN], F32, tag="ot", name="ot")
                    for tqb in range(NQB):
                        sl = slice(tqb * 512, (tqb + 1) * 512)
                        nc.vector.scalar_tensor_tensor(
                            ot[:, sl], ps[(o, tqb)][:], b2_c[oo][:],
                            x2_tiles[oo][:, sl].bitcast(F32),
                            op0=OP.add, op1=OP.add)
                    nc.sync.dma_start(out.ap()[oo * 128:(oo + 1) * 128, :], ot[:])


# ----------------------------------------------------------------------------
# host driver
# ----------------------------------------------------------------------------
B, S, E_FULL, H_FULL, MLP_FULL = 4, 2048, 1024, 16, 4096
_cache = {}


def _get_nc():
    if "nc" not in _cache:
        _cache["nc"] = build(Cfg())
    return _cache["nc"]


def _bf16(a):
    import ml_dtypes
    return np.ascontiguousarray(a.astype(ml_dtypes.bfloat16))


def _host_prepare(x_b, roll, qkv_w, fc_w, fc_b, ln1_g, ln1_b, ln2_g, ln2_b,
                  w1, b1, w2, b2):
    S_, E = x_b.shape
    D = 64
    xr = np.roll(x_b, -roll, axis=0)
    qkvT = np.ascontiguousarray(qkv_w.T).copy()
    qkvT[:, :E] *= D ** -0.5
    return {
        "xT": np.ascontiguousarray(xr.T),
        "qkvT": _bf16(qkvT),
        "fcT": _bf16(np.ascontiguousarray(fc_w.T)),
        "w1T": _bf16(np.ascontiguousarray(w1.T)),
        "w2T": _bf16(np.ascontiguousarray(w2.T)),
        "ln1": np.stack([ln1_g, ln1_b]).astype(np.float32),
        "ln2": np.stack([ln2_g, ln2_b]).astype(np.float32),
        "fcb": fc_b, "b1": b1, "b2": b2,
        "ones": np.ones((S_,), np.float32),
    }


def kernel(x, qkv_w, fc_w, fc_b, ln1_g, ln1_b, ln2_g, ln2_b, w1, b1, w2, b2):
    from concourse.bass_utils import run_bass_kernel_spmd

    x = np.ascontiguousarray(np.asarray(x, dtype=np.float32))
    args = [np.ascontiguousarray(np.asarray(a, dtype=np.float32)) for a in
            (qkv_w, fc_w, fc_b, ln1_g, ln1_b, ln2_g, ln2_b, w1, b1, w2, b2)]
    nc = _get_nc()
    in_maps = []
    for c in range(8):
        b, hf = c // 2, c % 2
        in_maps.append(_host_prepare(x[b], hf * (S // 2), *args))
    res = run_bass_kernel_spmd(nc, in_maps, list(range(8)))
    out = np.empty((B, S, E_FULL), np.float32)
    for c in range(8):
        b, hf = c // 2, c % 2
        out[b, hf * (S // 2):(hf + 1) * (S // 2), :] = res.results[c]["out"].T
    return out
